# revision 67
# baseline (speedup 1.0000x reference)
"""Causal multi-head attention on 8 Trainium2 NeuronCores.

Sharding: tensor-parallel across heads. 16 heads, 8 cores -> 2 heads/core.
Each core gets the full (pre-transposed, bf16) activations qT/kT/vT and its
slice of the projection weights; it computes the partial output
concat_c @ WoT_c [B*T, C]; the host sums the 8 partials (the "all-reduce
after the output projection").

Numerics: bf16 operands on the whole matmul path (per-element err ~0.4%
passes the 2e-2 gate with ~5x margin; fp8 anywhere fails it), fp32 PSUM
accumulation, softmax in fp32.  Measured end-to-end rel err ~4e-3.

Device math per core:
  KHT/QHT = W @ xT    [128 head-dims, 2048] bf16, 8 K-chunk accumulation
  VH      = xT.T @ Wv [128 tokens, 128 head-dims] per key block -- computed
            directly in key-major orientation (no PE transpose), with a
            ones column appended so the PV matmul also emits softmax
            denominators.
  Per (batch b, 512-wide query group qg), kb = key block, both heads l:
      ST[l]  = KH_l @ QHT_l block  [128 keys, 512-c0 queries] into paired
               PSUM banks (c0 = 128*d causal left-trim on diagonal blocks)
      P      = exp(ST/8)           ONE ACT per kb covering both heads
      causal: gpsimd.affine_select zeroes the 128-wide triangle block only
      OT[l] += [VH_l|1].T @ P[l]   [65, 512-c0] PSUM accumulate (partial-N
               first-touch relies on per-element has_written semantics)
    (PV lags the ST/exp chain by up to 6 kb so PE never stalls on
     ACT/Pool and chain-boundary otp-buffer WARs are absorbed)
    OT[0:64] *= 1/OT[64]  (DVE reciprocal + gpsimd partition_broadcast)
  OUT rows = OTall_b.T @ WoT_c, bf16 out.

Schedule notes (what the 248us -> 150us came from):
  - bf16 end-to-end halves DMA traffic (the v1 kernel was DMA-bound).
  - Input DMAs are 3-level-AP half-chunks (512KB each) on the SP queue
    only; out-DMAs are emitted after them so an out-DMA waiting on its
    evacuation copy never head-of-line-blocks input prefetch.  k/q chunks
    stream before v across the batch pair (STs need only k/q; the PV lag
    absorbs v landing later), and the very first k chunk arrives as
    quarters to shorten the cold start.
  - The softmax rescale of batch 1 is deferred into the next iteration
    (dependency-free by then); batch 0's runs immediately, while DVE has
    nothing pending.  Chain tails otherwise stall PE on PSUM evacuation.
  - outproj for group n runs at iteration n+1, before the attention
    chains, off the critical path.
  - PSUM budget (8 banks): ST pairs 2x2 + otp 2x1 + proj/outproj 2x1.
"""

import numpy as np

B, T, C = 2, 2048, 1024
H, DK = 16, 64
NCORES = 8
HL = H // NCORES          # local heads per core = 2
LD = HL * DK              # local head dims per core = 128
N = B * T                 # 4096 rows
KCH = C // 128            # 8 contraction chunks
QG = T // 512             # 4 query groups per batch
KB = T // 128             # 16 key blocks per batch

LAST_RESULTS = None       # BassKernelResults of the most recent run (for test.py)


def _build_program():
    import concourse.tile as tile
    import concourse.mybir as mybir
    from concourse import bacc
    from contextlib import ExitStack

    f32 = mybir.dt.float32
    bf16 = mybir.dt.bfloat16
    EXP = mybir.ActivationFunctionType.Exp

    nc = bacc.Bacc("TRN2", target_bir_lowering=False, debug=False, num_devices=NCORES)
    # activations pre-chunked on host: x[p, kk, t] = xT[kk*128+p, t]
    qT_d = nc.declare_dram_parameter("qT", [128, KCH, N], bf16, isOutput=False)
    kT_d = nc.declare_dram_parameter("kT", [128, KCH, N], bf16, isOutput=False)
    vT_d = nc.declare_dram_parameter("vT", [128, KCH, N], bf16, isOutput=False)
    # weights pre-chunked on host: w[p, kk, l] = WT[kk*128+p, l]
    wq_d = nc.declare_dram_parameter("wqT", [128, KCH, LD], bf16, isOutput=False)
    wk_d = nc.declare_dram_parameter("wkT", [128, KCH, LD], bf16, isOutput=False)
    wv_d = nc.declare_dram_parameter("wvT", [128, KCH, LD], bf16, isOutput=False)
    wo_d = nc.declare_dram_parameter("woT", [LD, C], bf16, isOutput=False)
    out_d = nc.declare_dram_parameter("out", [N, C], bf16, isOutput=True)

    with ExitStack() as ctx:
        tc = ctx.enter_context(tile.TileContext(nc))
        const = ctx.enter_context(tc.tile_pool(name="const", bufs=1))
        persist = ctx.enter_context(tc.tile_pool(name="persist", bufs=1))
        xpool = ctx.enter_context(tc.tile_pool(name="xt", bufs=4))
        ppool = ctx.enter_context(tc.tile_pool(name="p", bufs=10))
        opool = ctx.enter_context(tc.tile_pool(name="ot", bufs=4))
        spool = ctx.enter_context(tc.tile_pool(name="small", bufs=3))
        stps = ctx.enter_context(tc.tile_pool(name="stps", bufs=2, space="PSUM"))
        otps = ctx.enter_context(tc.tile_pool(name="otps", bufs=2, space="PSUM"))
        mps = ctx.enter_context(tc.tile_pool(name="mps", bufs=2, space="PSUM"))

        # ---- constants / weights: each weight's DMA is emitted just before
        # its first consumer so the first k-projection chunk doesn't queue
        # behind unrelated weight transfers on the DMA engines ----
        wq = const.tile([128, KCH, LD], bf16)
        wk = const.tile([128, KCH, LD], bf16)
        wv = const.tile([128, KCH, LD], bf16)
        wo = const.tile([128, C], bf16)
        w_dram_of = {id(wk): wk_d, id(wv): wv_d, id(wq): wq_d, id(wo): wo_d}
        w_loaded = set()

        def load_weight(w_t):
            if id(w_t) not in w_loaded:
                w_loaded.add(id(w_t))
                nc.sync.dma_start(w_t[:], w_dram_of[id(w_t)][:])

        # per-batch persistent activations
        qht = [persist.tile([128, T], bf16, name=f"qht{b}") for b in range(B)]
        kht = [persist.tile([128, T], bf16, name=f"kht{b}") for b in range(B)]
        otall = [persist.tile([128, T], bf16, name=f"otall{b}") for b in range(B)]
        # VH blocks [keys, dk] per (key block, local head), ones col appended
        vh = [persist.tile([128, KB, HL, 65], bf16, name=f"vh{b}") for b in range(B)]
        for b in range(B):
            nc.vector.memset(vh[b][:, :, :, 64:65], 1.0)

        loads = {}

        def _load(b, n, keys):
            # DMA issue only (SP queue), two half-DMAs per tensor; k/q are
            # streamed before v across the batch pair (STs need only k/q,
            # and the PV lag absorbs v arriving later)
            cols = slice(b * T + n * 512, b * T + (n + 1) * 512)
            tiles = loads.setdefault((b, n), {})
            for key, w_t, src in keys:
                load_weight(w_t)
                xh = [xpool.tile([128, KCH // 2, 512], bf16,
                                 name=f"xh_{key}{i}", tag=f"xh_{key}{i}")
                      for i in range(2)]
                for i in range(2):
                    nc.sync.dma_start(
                        xh[i][:], src[:, i * (KCH // 2):(i + 1) * (KCH // 2), cols])
                tiles[key] = xh

        def load_kq(b, n):
            if loads.get((b, n), {}).get("k", "x") is None:
                _load(b, n, (("q", wq, qT_d),))
            else:
                _load(b, n, (("k", wk, kT_d), ("q", wq, qT_d)))

        def load_v(b, n):
            _load(b, n, (("v", wv, vT_d),))

        def project_kq(b, n):
            # one 512-row group of the k/q projections (head dims on
            # partitions)
            tiles = loads[(b, n)]
            for key, w_t, dst in (("k", wk, kht[b]), ("q", wq, qht[b])):
                if key == "k" and tiles[key] is None:
                    ps = mps.tile([128, 512], f32, tag="mm")
                    for kk in range(KCH):
                        nc.tensor.matmul(ps[:], w_t[:, kk, :],
                                         kq0[kk // 2][:, kk % 2, :],
                                         start=(kk == 0), stop=(kk == KCH - 1))
                    nc.vector.tensor_copy(dst[:, 0:512], ps[:])
                    continue
                xh = tiles[key]
                ps = mps.tile([128, 512], f32, tag="mm")
                for kk in range(KCH):
                    nc.tensor.matmul(ps[:], w_t[:, kk, :],
                                     xh[kk // (KCH // 2)][:, kk % (KCH // 2), :],
                                     start=(kk == 0), stop=(kk == KCH - 1))
                nc.vector.tensor_copy(dst[:, n * 512:(n + 1) * 512], ps[:])
        def project_v(b, n):
            # v projection directly in key-major orientation (tokens on
            # partitions -- no PE transpose needed)
            # VH[tok, ld] = sum_c vT[c, tok] * WvT[c, ld], per key block
            vxh = loads.pop((b, n))["v"]
            for j in range(4):
                kb = 4 * n + j
                ps = mps.tile([128, 128], f32, tag="mm")
                for kk in range(KCH):
                    nc.tensor.matmul(
                        ps[:],
                        vxh[kk // (KCH // 2)][:, kk % (KCH // 2),
                                              j * 128:(j + 1) * 128],
                        wv[:, kk, :],
                        start=(kk == 0), stop=(kk == KCH - 1))
                nc.vector.tensor_copy(vh[b][:, kb, :, 0:64], ps[:])

        def attention_qg(b, qg, inject=()):
            q0 = qg * 512
            nkb = 4 * qg + 4
            lag = min(6, nkb)
            inject = dict(inject)
            otp = [otps.tile([65, 512], f32, tag="otp", name=f"otp_{b}_{qg}_{l}")
                   for l in range(HL)]
            ps = []

            def pv(kb):
                p, c0 = ps[kb]
                for l in range(HL):
                    nc.tensor.matmul(otp[l][:, c0:512], vh[b][:, kb, l, :],
                                     p[:, l, c0:512],
                                     start=(kb == 0), stop=(kb == nkb - 1))

            for kb in range(nkb):
                d = kb - 4 * qg
                c0 = 128 * d if d > 0 else 0
                st = stps.tile([128, HL, 512], f32, tag="st",
                               name=f"st_{b}_{qg}_{kb}")
                for l in range(HL):
                    hs = slice(l * 64, (l + 1) * 64)
                    nc.tensor.matmul(
                        st[:, l, c0:512],
                        kht[b][hs, kb * 128:(kb + 1) * 128],
                        qht[b][hs, q0 + c0: q0 + 512],
                        start=True, stop=True)
                p = ppool.tile([128, HL, 512], bf16, tag="p",
                               name=f"p_{b}_{qg}_{kb}")
                nc.scalar.activation(p[:, :, c0:512], st[:, :, c0:512],
                                     EXP, scale=0.125)
                if d >= 0:
                    # zero keys below the causal diagonal; only the 128-wide
                    # triangle block [c0:c0+128] can violate causality (with
                    # j = col - c0 and c0 = 128*d, keep where j >= key
                    # partition); columns beyond it are fully valid
                    nc.gpsimd.affine_select(
                        out=p[:, :, c0:c0 + 128], in_=p[:, :, c0:c0 + 128],
                        compare_op=mybir.AluOpType.is_ge,
                        fill=0.0, base=0, channel_multiplier=-1,
                        pattern=[[0, HL], [1, 128]])
                ps.append((p, c0))
                if kb >= lag:
                    pv(kb - lag)
                fn = inject.pop(kb, None)
                if fn is not None:
                    fn()
            for t in range(lag, 0, -1):
                pv(nkb - t)

            def rescale():
                # deferred to the next iteration: by then the otp data is
                # long-ready, so these never head-of-line-block the DVE queue
                for l in range(HL):
                    recip = spool.tile([1, 512], f32, tag="recip")
                    nc.vector.reciprocal(recip[:], otp[l][64:65, :])
                    rep = spool.tile([64, 512], f32, tag="rep")
                    nc.gpsimd.partition_broadcast(rep[:], recip[:])
                    with nc.allow_low_precision(reason="bf16 out of f32 softmax"):
                        nc.vector.tensor_mul(
                            otall[b][l * 64:(l + 1) * 64, q0: q0 + 512],
                            otp[l][0:64, :], rep[:])
            return rescale

        def outproj_qg(b, qg, tail=False):
            # output projection + store for this 512-row group; evacuation
            # copies alternate DVE/ACT so neither queue head-of-line blocks
            # (in the drain tail DVE is stuck behind the final softmax
            # rescale, so everything goes to ACT there); each half is DMA'd
            # as soon as its copy lands
            load_weight(wo)
            q0 = qg * 512
            # the out-DMA is issued from the same queue as the evacuation
            # copy: it needs no extra semaphore wait there, and keeps the SP
            # queue free for input prefetch (an out-DMA waiting on its copy
            # otherwise blocks all later input DMAs behind it)
            for rt in range(4):
                use_act = tail and (b == 0 or rt % 2 == 1)
                row0 = q0 + rt * 128
                ot = opool.tile([128, 2, 512], bf16, tag="ot")
                for nn in range(2):
                    ops = mps.tile([128, 512], f32, tag="mm")
                    nc.tensor.matmul(ops[:], otall[b][:, row0:row0 + 128],
                                     wo[:, nn * 512:(nn + 1) * 512],
                                     start=True, stop=True)
                    if use_act:
                        nc.scalar.copy(ot[:, nn, :], ops[:])
                    else:
                        nc.vector.tensor_copy(ot[:, nn, :], ops[:])
                eng = nc.scalar if use_act else nc.sync
                eng.dma_start(
                    out_d[b * T + row0: b * T + row0 + 128, :], ot[:])

        # group-granular interleave; rescale + outproj are deferred one
        # iteration so their DVE work is dependency-free when the queue
        # reaches it (emitted at the tail of a chain they head-of-line-block
        # DVE on the last PV, stalling PE on PSUM evacuations)
        # Rescale placement: b0's runs immediately after its chain (DVE has
        # nothing pending during b1's chain, so the PV-end wait is harmless);
        # b1's is deferred into the next iteration, where its inputs are
        # long-ready — so it neither blocks the DVE queue nor outlives its
        # otp buffers (the 2-deep otp pool is only recycled by the next
        # iteration's chains, after the deferred rescale has run).
        r_b1 = None
        # cold start: the very first k chunk arrives as quarters so the
        # first projection chain starts ~1.5us earlier and stays fed
        cols0 = slice(0, 512)
        load_weight(wk)
        kq0 = [xpool.tile([128, 2, 512], bf16, name=f"kq0_{i}",
                          tag=f"kq0_{i}", bufs=1) for i in range(4)]
        for i in range(4):
            nc.sync.dma_start(kq0[i][:], kT_d[:, 2 * i:2 * i + 2, cols0])
        loads[(0, 0)] = {"k": None}
        load_kq(0, 0)
        load_kq(1, 0)
        load_v(0, 0)
        load_v(1, 0)
        for n in range(QG):
            project_kq(0, n)
            if r_b1 is not None:
                r_b1()
            project_kq(1, n)
            project_v(0, n)
            project_v(1, n)
            # the input stream owns the DMA engines mid-run (they are
            # saturated through the first two thirds of the run); ALL output
            # projections are deferred to the endgame, where the last
            # attention chain is ACT-bound and both PE and the DMA engines
            # have slack
            if n + 1 < QG:
                load_kq(0, n + 1)
                load_kq(1, n + 1)
                load_v(0, n + 1)
                load_v(1, n + 1)
            r_b0 = attention_qg(0, n)
            r_b0()
            if n == QG - 1:
                # batch 0's four outprojs interleave into batch 1's final
                # (longest, ACT-paced) chain, where PE and DVE have slack
                inj = tuple((4 + 3 * qg, (lambda qg=qg: outproj_qg(0, qg)))
                            for qg in range(QG))
                r_b1 = attention_qg(1, n, inject=inj)
            else:
                r_b1 = attention_qg(1, n)
        r_b1()
        for qg in range(QG):
            outproj_qg(1, qg, tail=True)

    nc.compile()
    return nc


def kernel(q, k, v, Wq, Wk, Wv, Wo):
    global LAST_RESULTS
    import ml_dtypes
    from concourse.bass_utils import run_bass_kernel_spmd

    bf16 = ml_dtypes.bfloat16

    def chunk_T(x):
        # [N, C] -> xT [C, N] -> [128, KCH, N] with x[p, kk, t] = xT[kk*128+p, t]
        xT = np.asarray(x, np.float32).reshape(N, C).T
        return np.ascontiguousarray(
            xT.reshape(KCH, 128, N).transpose(1, 0, 2)).astype(bf16)

    qc, kc, vc = chunk_T(q), chunk_T(k), chunk_T(v)
    Wq = np.asarray(Wq, np.float32)
    Wk = np.asarray(Wk, np.float32)
    Wv = np.asarray(Wv, np.float32)
    Wo = np.asarray(Wo, np.float32)

    def chunk_W(W, sl):
        # Wc = W[sl, :].T [C, LD] -> [128, KCH, LD]
        WT = W[sl, :].T
        return np.ascontiguousarray(
            WT.reshape(KCH, 128, LD).transpose(1, 0, 2)).astype(bf16)

    in_maps = []
    for c in range(NCORES):
        sl = slice(c * LD, (c + 1) * LD)
        in_maps.append({
            "qT": qc, "kT": kc, "vT": vc,
            "wqT": chunk_W(Wq, sl),
            "wkT": chunk_W(Wk, sl),
            "wvT": chunk_W(Wv, sl),
            "woT": np.ascontiguousarray(Wo[:, sl].T).astype(bf16),
        })

    nc = _build_program()
    res = run_bass_kernel_spmd(nc, in_maps, list(range(NCORES)))
    LAST_RESULTS = res
    acc = np.zeros((N, C), np.float32)
    for rmap in res.results:
        acc += np.asarray(rmap["out"], np.float32)
    return acc.reshape(B, T, C)


# revision 68
# speedup vs baseline: 1.0292x; 1.0292x over previous
"""Causal multi-head attention on 8 Trainium2 NeuronCores.

Sharding: tensor-parallel across heads. 16 heads, 8 cores -> 2 heads/core.
Each core gets the full (pre-transposed, bf16) activations qT/kT/vT and its
slice of the projection weights; it computes the partial output
concat_c @ WoT_c [B*T, C]; the host sums the 8 partials (the "all-reduce
after the output projection").

Numerics: bf16 operands on the whole matmul path (per-element err ~0.4%
passes the 2e-2 gate with ~5x margin; fp8 anywhere fails it), fp32 PSUM
accumulation, softmax in fp32.  Measured end-to-end rel err ~4e-3.

Device math per core:
  KHT/QHT = W @ xT    [128 head-dims, 2048] bf16, 8 K-chunk accumulation
  VH      = xT.T @ Wv [128 tokens, 128 head-dims] per key block -- computed
            directly in key-major orientation (no PE transpose), with a
            ones column appended so the PV matmul also emits softmax
            denominators.
  Per (batch b, 512-wide query group qg), kb = key block, both heads l:
      ST[l]  = KH_l @ QHT_l block  [128 keys, 512-c0 queries] into paired
               PSUM banks (c0 = 128*d causal left-trim on diagonal blocks)
      P      = exp(ST/8)           ONE ACT per kb covering both heads
      causal: gpsimd.affine_select zeroes the 128-wide triangle block only
      OT[l] += [VH_l|1].T @ P[l]   [65, 512-c0] PSUM accumulate (partial-N
               first-touch relies on per-element has_written semantics)
    (PV lags the ST/exp chain by up to 6 kb so PE never stalls on
     ACT/Pool and chain-boundary otp-buffer WARs are absorbed)
    OT[0:64] *= 1/OT[64]  (DVE reciprocal + gpsimd partition_broadcast)
  OUT rows = OTall_b.T @ WoT_c, bf16 out.

Schedule notes (what the 248us -> 150us came from):
  - bf16 end-to-end halves DMA traffic (the v1 kernel was DMA-bound).
  - Input DMAs are 3-level-AP half-chunks (512KB each) on the SP queue
    only; out-DMAs are emitted after them so an out-DMA waiting on its
    evacuation copy never head-of-line-blocks input prefetch.  k/q chunks
    stream before v across the batch pair (STs need only k/q; the PV lag
    absorbs v landing later), and the very first k chunk arrives as
    quarters to shorten the cold start.
  - The softmax rescale of batch 1 is deferred into the next iteration
    (dependency-free by then); batch 0's runs immediately, while DVE has
    nothing pending.  Chain tails otherwise stall PE on PSUM evacuation.
  - outproj for group n runs at iteration n+1, before the attention
    chains, off the critical path.
  - PSUM budget (8 banks): ST pairs 2x2 + otp 2x1 + proj/outproj 2x1.
"""

import numpy as np

B, T, C = 2, 2048, 1024
H, DK = 16, 64
NCORES = 8
HL = H // NCORES          # local heads per core = 2
LD = HL * DK              # local head dims per core = 128
N = B * T                 # 4096 rows
KCH = C // 128            # 8 contraction chunks
QG = T // 512             # 4 query groups per batch
KB = T // 128             # 16 key blocks per batch

LAST_RESULTS = None       # BassKernelResults of the most recent run (for test.py)


def _build_program():
    import concourse.tile as tile
    import concourse.mybir as mybir
    from concourse import bacc
    from contextlib import ExitStack

    f32 = mybir.dt.float32
    bf16 = mybir.dt.bfloat16
    EXP = mybir.ActivationFunctionType.Exp

    nc = bacc.Bacc("TRN2", target_bir_lowering=False, debug=False, num_devices=NCORES)
    # activations pre-chunked on host: x[p, kk, t] = xT[kk*128+p, t]
    qT_d = nc.declare_dram_parameter("qT", [128, KCH, N], bf16, isOutput=False)
    kT_d = nc.declare_dram_parameter("kT", [128, KCH, N], bf16, isOutput=False)
    vT_d = nc.declare_dram_parameter("vT", [128, KCH, N], bf16, isOutput=False)
    # weights pre-chunked on host: w[p, kk, l] = WT[kk*128+p, l]
    wq_d = nc.declare_dram_parameter("wqT", [128, KCH, LD], bf16, isOutput=False)
    wk_d = nc.declare_dram_parameter("wkT", [128, KCH, LD], bf16, isOutput=False)
    wv_d = nc.declare_dram_parameter("wvT", [128, KCH, LD], bf16, isOutput=False)
    wo_d = nc.declare_dram_parameter("woT", [LD, C], bf16, isOutput=False)
    out_d = nc.declare_dram_parameter("out", [N, C], bf16, isOutput=True)

    with ExitStack() as ctx:
        tc = ctx.enter_context(tile.TileContext(nc))
        const = ctx.enter_context(tc.tile_pool(name="const", bufs=1))
        persist = ctx.enter_context(tc.tile_pool(name="persist", bufs=1))
        xpool = ctx.enter_context(tc.tile_pool(name="xt", bufs=4))
        ppool = ctx.enter_context(tc.tile_pool(name="p", bufs=10))
        opool = ctx.enter_context(tc.tile_pool(name="ot", bufs=4))
        spool = ctx.enter_context(tc.tile_pool(name="small", bufs=3))
        stps = ctx.enter_context(tc.tile_pool(name="stps", bufs=2, space="PSUM"))
        otps = ctx.enter_context(tc.tile_pool(name="otps", bufs=2, space="PSUM"))
        mps = ctx.enter_context(tc.tile_pool(name="mps", bufs=2, space="PSUM"))

        # ---- constants / weights: each weight's DMA is emitted just before
        # its first consumer so the first k-projection chunk doesn't queue
        # behind unrelated weight transfers on the DMA engines ----
        wq = const.tile([128, KCH, LD], bf16)
        wk = const.tile([128, KCH, LD], bf16)
        wv = const.tile([128, KCH, LD], bf16)
        wo = const.tile([128, C], bf16)
        w_dram_of = {id(wk): wk_d, id(wv): wv_d, id(wq): wq_d, id(wo): wo_d}
        w_loaded = set()

        def load_weight(w_t):
            if id(w_t) not in w_loaded:
                w_loaded.add(id(w_t))
                nc.sync.dma_start(w_t[:], w_dram_of[id(w_t)][:])

        # per-batch persistent activations
        qht = [persist.tile([128, T], bf16, name=f"qht{b}") for b in range(B)]
        kht = [persist.tile([128, T], bf16, name=f"kht{b}") for b in range(B)]
        otall = [persist.tile([128, T], bf16, name=f"otall{b}") for b in range(B)]
        # VH blocks [keys, dk] per (key block, local head), ones col appended
        vh = [persist.tile([128, KB, HL, 65], bf16, name=f"vh{b}") for b in range(B)]
        for b in range(B):
            nc.vector.memset(vh[b][:, :, :, 64:65], 1.0)

        loads = {}

        def _load(b, n, keys):
            # DMA issue only (SP queue), two half-DMAs per tensor; k/q are
            # streamed before v across the batch pair (STs need only k/q,
            # and the PV lag absorbs v arriving later)
            cols = slice(b * T + n * 512, b * T + (n + 1) * 512)
            tiles = loads.setdefault((b, n), {})
            for key, w_t, src in keys:
                load_weight(w_t)
                xh = [xpool.tile([128, KCH // 2, 512], bf16,
                                 name=f"xh_{key}{i}", tag=f"xh_{key}{i}")
                      for i in range(2)]
                for i in range(2):
                    nc.sync.dma_start(
                        xh[i][:], src[:, i * (KCH // 2):(i + 1) * (KCH // 2), cols])
                tiles[key] = xh

        def load_kq(b, n):
            if loads.get((b, n), {}).get("k", "x") is None:
                _load(b, n, (("q", wq, qT_d),))
            else:
                _load(b, n, (("k", wk, kT_d), ("q", wq, qT_d)))

        def load_v(b, n):
            _load(b, n, (("v", wv, vT_d),))

        def project_kq(b, n):
            # one 512-row group of the k/q projections (head dims on
            # partitions)
            tiles = loads[(b, n)]
            for key, w_t, dst in (("k", wk, kht[b]), ("q", wq, qht[b])):
                if key == "k" and tiles[key] is None:
                    ps = mps.tile([128, 512], f32, tag="mm")
                    for kk in range(KCH):
                        nc.tensor.matmul(ps[:], w_t[:, kk, :],
                                         kq0[kk // 2][:, kk % 2, :],
                                         start=(kk == 0), stop=(kk == KCH - 1))
                    nc.vector.tensor_copy(dst[:, 0:512], ps[:])
                    continue
                xh = tiles[key]
                ps = mps.tile([128, 512], f32, tag="mm")
                for kk in range(KCH):
                    nc.tensor.matmul(ps[:], w_t[:, kk, :],
                                     xh[kk // (KCH // 2)][:, kk % (KCH // 2), :],
                                     start=(kk == 0), stop=(kk == KCH - 1))
                nc.vector.tensor_copy(dst[:, n * 512:(n + 1) * 512], ps[:])
        def project_v(b, n):
            # v projection directly in key-major orientation (tokens on
            # partitions -- no PE transpose needed)
            # VH[tok, ld] = sum_c vT[c, tok] * WvT[c, ld], per key block
            vxh = loads.pop((b, n))["v"]
            for j in range(4):
                kb = 4 * n + j
                ps = mps.tile([128, 128], f32, tag="mm")
                for kk in range(KCH):
                    nc.tensor.matmul(
                        ps[:],
                        vxh[kk // (KCH // 2)][:, kk % (KCH // 2),
                                              j * 128:(j + 1) * 128],
                        wv[:, kk, :],
                        start=(kk == 0), stop=(kk == KCH - 1))
                nc.vector.tensor_copy(vh[b][:, kb, :, 0:64], ps[:])

        def attention_qg(b, qg, inject=()):
            q0 = qg * 512
            nkb = 4 * qg + 4
            lag = min(6, nkb)
            inject = dict(inject)
            otp = [otps.tile([65, 512], f32, tag="otp", name=f"otp_{b}_{qg}_{l}")
                   for l in range(HL)]
            ps = []

            def pv(kb):
                p, c0 = ps[kb]
                for l in range(HL):
                    nc.tensor.matmul(otp[l][:, c0:512], vh[b][:, kb, l, :],
                                     p[:, l, c0:512],
                                     start=(kb == 0), stop=(kb == nkb - 1))

            for kb in range(nkb):
                d = kb - 4 * qg
                c0 = 128 * d if d > 0 else 0
                st = stps.tile([128, HL, 512], f32, tag="st",
                               name=f"st_{b}_{qg}_{kb}")
                for l in range(HL):
                    hs = slice(l * 64, (l + 1) * 64)
                    nc.tensor.matmul(
                        st[:, l, c0:512],
                        kht[b][hs, kb * 128:(kb + 1) * 128],
                        qht[b][hs, q0 + c0: q0 + 512],
                        start=True, stop=True)
                p = ppool.tile([128, HL, 512], bf16, tag="p",
                               name=f"p_{b}_{qg}_{kb}")
                nc.scalar.activation(p[:, :, c0:512], st[:, :, c0:512],
                                     EXP, scale=0.125)
                if d >= 0:
                    # zero keys below the causal diagonal; only the 128-wide
                    # triangle block [c0:c0+128] can violate causality (with
                    # j = col - c0 and c0 = 128*d, keep where j >= key
                    # partition); columns beyond it are fully valid
                    nc.gpsimd.affine_select(
                        out=p[:, :, c0:c0 + 128], in_=p[:, :, c0:c0 + 128],
                        compare_op=mybir.AluOpType.is_ge,
                        fill=0.0, base=0, channel_multiplier=-1,
                        pattern=[[0, HL], [1, 128]])
                ps.append((p, c0))
                if kb >= lag:
                    pv(kb - lag)
                fn = inject.pop(kb, None)
                if fn is not None:
                    fn()
            for t in range(lag, 0, -1):
                pv(nkb - t)

            def rescale():
                # deferred to the next iteration: by then the otp data is
                # long-ready, so these never head-of-line-block the DVE queue
                for l in range(HL):
                    recip = spool.tile([1, 512], f32, tag="recip")
                    nc.vector.reciprocal(recip[:], otp[l][64:65, :])
                    rep = spool.tile([64, 512], f32, tag="rep")
                    nc.gpsimd.partition_broadcast(rep[:], recip[:])
                    with nc.allow_low_precision(reason="bf16 out of f32 softmax"):
                        nc.vector.tensor_mul(
                            otall[b][l * 64:(l + 1) * 64, q0: q0 + 512],
                            otp[l][0:64, :], rep[:])
            return rescale

        def outproj_qg(b, qg, tail=False):
            # output projection + store for this 512-row group; evacuation
            # copies alternate DVE/ACT so neither queue head-of-line blocks
            # (in the drain tail DVE is stuck behind the final softmax
            # rescale, so everything goes to ACT there); each half is DMA'd
            # as soon as its copy lands
            load_weight(wo)
            q0 = qg * 512
            # the out-DMA is issued from the same queue as the evacuation
            # copy: it needs no extra semaphore wait there, and keeps the SP
            # queue free for input prefetch (an out-DMA waiting on its copy
            # otherwise blocks all later input DMAs behind it)
            for rt in range(4):
                use_act = tail and (b == 0 or rt % 2 == 1)
                row0 = q0 + rt * 128
                ot = opool.tile([128, 2, 512], bf16, tag="ot")
                for nn in range(2):
                    ops = mps.tile([128, 512], f32, tag="mm")
                    nc.tensor.matmul(ops[:], otall[b][:, row0:row0 + 128],
                                     wo[:, nn * 512:(nn + 1) * 512],
                                     start=True, stop=True)
                    if use_act:
                        nc.scalar.copy(ot[:, nn, :], ops[:])
                    else:
                        nc.vector.tensor_copy(ot[:, nn, :], ops[:])
                eng = nc.scalar if use_act else nc.sync
                eng.dma_start(
                    out_d[b * T + row0: b * T + row0 + 128, :], ot[:])

        # group-granular interleave; rescale + outproj are deferred one
        # iteration so their DVE work is dependency-free when the queue
        # reaches it (emitted at the tail of a chain they head-of-line-block
        # DVE on the last PV, stalling PE on PSUM evacuations)
        # Rescale placement: b0's runs immediately after its chain (DVE has
        # nothing pending during b1's chain, so the PV-end wait is harmless);
        # b1's is deferred into the next iteration, where its inputs are
        # long-ready — so it neither blocks the DVE queue nor outlives its
        # otp buffers (the 2-deep otp pool is only recycled by the next
        # iteration's chains, after the deferred rescale has run).
        r_b1 = None
        # cold start: the very first k chunk arrives as quarters so the
        # first projection chain starts ~1.5us earlier and stays fed
        cols0 = slice(0, 512)
        load_weight(wk)
        kq0 = [xpool.tile([128, 2, 512], bf16, name=f"kq0_{i}",
                          tag=f"kq0_{i}", bufs=1) for i in range(4)]
        for i in range(4):
            nc.sync.dma_start(kq0[i][:], kT_d[:, 2 * i:2 * i + 2, cols0])
        loads[(0, 0)] = {"k": None}
        load_kq(0, 0)
        load_kq(1, 0)
        load_v(0, 0)
        load_v(1, 0)
        for n in range(QG):
            project_kq(0, n)
            if r_b1 is not None:
                r_b1()
            project_kq(1, n)
            project_v(0, n)
            project_v(1, n)
            # the input stream owns the DMA engines through the first two
            # thirds of the run, so outprojs for qg 0/1 are deferred to the
            # endgame (where the last attention chain is ACT-bound and PE,
            # DVE and the DMA engines all have slack); qg 2's runs mid-loop
            # once the input stream has drained
            if n == QG - 1:
                for b in range(B):
                    outproj_qg(b, QG - 2)
            if n + 1 < QG:
                load_kq(0, n + 1)
                load_kq(1, n + 1)
                load_v(0, n + 1)
                load_v(1, n + 1)
            r_b0 = attention_qg(0, n)
            r_b0()
            if n == QG - 1:
                # batch 0's deferred outprojs interleave into batch 1's
                # final (longest, ACT-paced) chain
                inj = tuple((4 + 4 * i, (lambda qg=qg: outproj_qg(0, qg)))
                            for i, qg in enumerate((0, 1, QG - 1)))
                r_b1 = attention_qg(1, n, inject=inj)
            else:
                r_b1 = attention_qg(1, n)
        r_b1()
        for qg in (0, 1, QG - 1):
            outproj_qg(1, qg, tail=True)

    nc.compile()
    return nc


def kernel(q, k, v, Wq, Wk, Wv, Wo):
    global LAST_RESULTS
    import ml_dtypes
    from concourse.bass_utils import run_bass_kernel_spmd

    bf16 = ml_dtypes.bfloat16

    def chunk_T(x):
        # [N, C] -> xT [C, N] -> [128, KCH, N] with x[p, kk, t] = xT[kk*128+p, t]
        xT = np.asarray(x, np.float32).reshape(N, C).T
        return np.ascontiguousarray(
            xT.reshape(KCH, 128, N).transpose(1, 0, 2)).astype(bf16)

    qc, kc, vc = chunk_T(q), chunk_T(k), chunk_T(v)
    Wq = np.asarray(Wq, np.float32)
    Wk = np.asarray(Wk, np.float32)
    Wv = np.asarray(Wv, np.float32)
    Wo = np.asarray(Wo, np.float32)

    def chunk_W(W, sl):
        # Wc = W[sl, :].T [C, LD] -> [128, KCH, LD]
        WT = W[sl, :].T
        return np.ascontiguousarray(
            WT.reshape(KCH, 128, LD).transpose(1, 0, 2)).astype(bf16)

    in_maps = []
    for c in range(NCORES):
        sl = slice(c * LD, (c + 1) * LD)
        in_maps.append({
            "qT": qc, "kT": kc, "vT": vc,
            "wqT": chunk_W(Wq, sl),
            "wkT": chunk_W(Wk, sl),
            "wvT": chunk_W(Wv, sl),
            "woT": np.ascontiguousarray(Wo[:, sl].T).astype(bf16),
        })

    nc = _build_program()
    res = run_bass_kernel_spmd(nc, in_maps, list(range(NCORES)))
    LAST_RESULTS = res
    acc = np.zeros((N, C), np.float32)
    for rmap in res.results:
        acc += np.asarray(rmap["out"], np.float32)
    return acc.reshape(B, T, C)
